# revision 5
# baseline (speedup 1.0000x reference)
"""Trainium2 Bass kernel for SinusoidalEncoder.

Reference (per element):
  out[b, s, 2i]   = sin(x[b, s, 0] * f_i)
  out[b, s, 2i+1] = cos(x[b, s, 1] * f_i),  f_i = 2^(2i/256 * 10)

Sharding: pure data-parallel over batch (4 batches per core on 8 cores).
Per core 32768 tokens, laid out p-major (partition p owns tokens
p*256 + jc), so x loads and output stores are large per-partition
contiguous DMAs.

Math (in "turns"): u = x * (f_i/2pi), +1/4 turn on cos slots.
Magic-constant range reduction, M = 1.5*2^23:
  k  = fl(u + M) - M          (integer nearest u)
  -r = k - u  in [-0.5, 0.5]
  out = sin(-2pi * -r)

Per chunk [128, 2048] (8 token-groups x 256 interleaved slots):
  1. DVE  tensor_tensor (stride-0 broadcast APs):
       U[p, jj, i, b] = F2INT[p, i, b] * X[p, jj, b]     (one op)
  2. ACT  Identity strided over odd slots: U_odd += 0.25  (in place)
  3. ACT  Identity: UM = fl(U + M)
  4. DVE  scalar_tensor_tensor: R = (UM - M) - U  = -r    (one op)
  5. ACT  Sin: O = sin(R * -2pi)
  6. HWDGE store (1MB, 8KB/partition contiguous)
"""
import numpy as np

_NCORES = 8
_B, _S, _C = 32, 8192, 3
_D = 256
_NTOK = (_B // _NCORES) * _S          # 32768 tokens per core
_JC = _NTOK // 128                    # 256 token-groups per core
_K = 16                               # token-groups per chunk
_NCHUNK = _JC // _K                   # 32 chunks
_CW = _D * _K                         # 2048 floats per chunk
_MAGIC = 12582912.0                   # 1.5 * 2^23
_TWO_PI = float(2 * np.pi)

_cache = {}


def _freqs_half():
    e = np.arange(0, _D, 2, dtype=np.float64) / _D * 10.0
    return (np.power(2.0, e) / (2 * np.pi)).astype(np.float32)  # [128]


def _build():
    import concourse.bacc as bacc
    import concourse.tile as tile
    import concourse.mybir as mybir

    F32 = mybir.dt.float32
    AF = mybir.ActivationFunctionType
    ALU = mybir.AluOpType

    nc = bacc.Bacc("TRN2", target_bir_lowering=False, debug=False)
    x_d = nc.dram_tensor("x", [_NTOK, _C], F32, kind="ExternalInput")
    f2_d = nc.dram_tensor("f2i", [128, _D], F32, kind="ExternalInput")
    out_d = nc.dram_tensor("out", [_NTOK, _D], F32, kind="ExternalOutput")

    x_view = x_d.ap().rearrange("(p f) c -> p (f c)", p=128)      # [128, 768]
    out_view = out_d.ap().rearrange("(p n) d -> p (n d)", p=128)  # [128, 65536]

    with tile.TileContext(nc) as tc:
        with (
            tc.tile_pool(name="const", bufs=1) as cpool,
            tc.tile_pool(name="u", bufs=3) as upool,
            tc.tile_pool(name="um", bufs=2) as umpool,
            tc.tile_pool(name="r", bufs=2) as rpool,
            tc.tile_pool(name="o", bufs=3) as opool,
        ):
            MB = cpool.tile([128, 1], F32, tag="MB")
            nc.gpsimd.memset(MB[:], _MAGIC)
            QB = cpool.tile([128, 1], F32, tag="QB")
            nc.gpsimd.memset(QB[:], 0.25)
            X = cpool.tile([128, 3 * _JC], F32, tag="X")
            nc.sync.dma_start(X[:], x_view)
            F2 = cpool.tile([128, _D], F32, tag="F2")
            nc.sync.dma_start(F2[:], f2_d.ap())

            # variable chunk sizes: small ramp-up/ramp-down, big middle
            sizes = [1024, 1024, 2048] + [4096] * 14 + [2048, 1024, 1024]
            assert sum(sizes) == _JC * _D
            # chunks whose magic-add runs on DVE instead of ACT (balance)
            dve_um = {5}

            col = 0
            for c, cw in enumerate(sizes):
                k = cw // _D  # token-groups this chunk
                j0 = col // _D
                f2_b = (
                    F2[:]
                    .rearrange("p (i b) -> p i b", b=2)
                    .unsqueeze(1)
                    .broadcast_to((128, k, 128, 2))
                )
                # 1. phase: U[p, jj, i, b] = f2_i * x_b(token j0+jj)
                U = upool.tile([128, cw], F32, tag="U")
                u4 = U[:].rearrange("p (jj i b) -> p jj i b", jj=k, b=2)
                x_b = (
                    X[:, 3 * j0 : 3 * (j0 + k)]
                    .rearrange("p (jj b) -> p jj b", b=3)[:, :, 0:2]
                    .unsqueeze(2)
                    .broadcast_to((128, k, 128, 2))
                )
                nc.vector.tensor_tensor(u4, f2_b, x_b, ALU.mult)

                # 2. quarter turn on odd (cos) slots, in place
                u_odd = u4[:, :, :, 1:2]
                nc.scalar.activation(u_odd, u_odd, AF.Identity, bias=QB[:, 0:1])

                # 3. magic add: UM = fl(U + M) = k + M
                UM = umpool.tile([128, cw], F32, tag="UM")
                if c in dve_um:
                    nc.vector.tensor_scalar(UM[:], U[:], _MAGIC, None, ALU.add)
                else:
                    nc.scalar.activation(UM[:], U[:], AF.Identity, bias=MB[:, 0:1])

                # 4. fused: R = (UM - M) - U = k - u = -r
                R = rpool.tile([128, cw], F32, tag="R")
                nc.vector.scalar_tensor_tensor(
                    R[:], UM[:], _MAGIC, U[:], ALU.subtract, ALU.subtract
                )

                # 5. sin: O = sin(-2pi * R) = sin(2pi r)
                O = opool.tile([128, cw], F32, tag="O")
                nc.scalar.activation(O[:], R[:], AF.Sin, scale=-_TWO_PI)

                # 6. store
                nc.sync.dma_start(out_view[:, col : col + cw], O[:])
                col += cw

    nc.compile()
    return nc


def _get_nc():
    if "nc" not in _cache:
        _cache["nc"] = _build()
    return _cache["nc"]


def _f2i_input():
    f2 = _freqs_half()
    f2i = np.empty(_D, np.float32)
    f2i[0::2] = f2
    f2i[1::2] = f2
    return np.tile(f2i[None, :], (128, 1))


def kernel(x: np.ndarray) -> np.ndarray:
    from concourse.bass_utils import run_bass_kernel_spmd

    nc = _get_nc()
    x = np.ascontiguousarray(np.asarray(x, dtype=np.float32))
    f2i = _f2i_input()
    shards = x.reshape(_NCORES, _NTOK, _C)
    in_maps = [{"x": shards[i], "f2i": f2i} for i in range(_NCORES)]
    res = run_bass_kernel_spmd(nc, in_maps, list(range(_NCORES)))
    out = np.stack([res.results[i]["out"] for i in range(_NCORES)])
    return out.reshape(_B, _S, _D)
